# revision 29
# baseline (speedup 1.0000x reference)
"""Trainium2 Bass kernel for nn_Actor (segmented categorical sampling head).

Math (matches reference):
  h      = tanh(inputs @ W1 + b1)                    [1,128]
  scores = 10*tanh(h @ W2 + b2)                      [200000]
  logits = scores.reshape(2000, 100)
  G      = jax.random.gumbel(key(42), (128,2000,100), f32)   (input-independent)
  positions[s,n] = argmax_j(logits[n,j] + G[s,n,j])
  log_softmax[s] = sum_n log_softmax(logits)[n, positions[s,n]]

Sharding: 8-way tensor parallel over all_node_num (25000 scores = 250
segments per core). W1/inputs are sharded over input_dim with an on-device
AllReduce of the partial h. Each core computes its local scores + samples
its local segments; host concatenates positions/scores and sums log-probs.

Device encoding trick: per (sample, segment) the kernel emits
  v = lsm[pos] + 1000*pos
via blocked DVE/GpSimd ops: tmp = G+logits (broadcast AP over samples),
mx = max_j tmp, eq = (tmp == mx) (broadcast over j), prod = eq*comb
(comb = lsm + 1000*j), v = sum_j prod. Host decodes pos = round(v/1000)
exactly and lsm[pos] = v - 1000*pos.

G must be generated on the default jax backend — the same backend the
reference's jax.random.categorical runs on (the env's default PRNG impl
"rbg" is backend-dependent).
"""

import os
import numpy as np

import concourse.bass as bass
import concourse.bacc as bacc
import concourse.tile as tile
from concourse import mybir
from concourse.bass_utils import run_bass_kernel_spmd

F32 = mybir.dt.float32
AX = mybir.AxisListType
ALU = mybir.AluOpType
ACTF = mybir.ActivationFunctionType

N_CORES = 8
INPUT_DIM = 16384
HIDDEN = 128
ALL_NODE = 200000
NODE = 2000
SEG = 100
SAMPLE = 128
C_SCALE = 10.0

NPC = ALL_NODE // N_CORES        # 25000 scores per core
SEGS_PC = NODE // N_CORES        # 250 segments per core
TILE_P = 125                     # segment partitions per tile
NTILES = SEGS_PC // TILE_P       # 2
SBLK = 32                        # samples per G block
NBLK = SAMPLE // SBLK            # 4
KPC = INPUT_DIM // N_CORES       # 2048 input rows per core
NK = KPC // 128                  # 16 k-chunks per core
NKF = INPUT_DIM // 128           # 128 k-chunks (replicated W1)
W1_GRP = 8                       # chunks per W1 DMA group
N_W1_GRP = NKF // W1_GRP         # 16
MM2 = 500                        # w2 columns per matmul (= 5 segments)
NCH = NPC // MM2                 # 50 chunks; 25 per tile
W2_TILE = 2500                   # w2 columns per DMA tile
N_W2_TILE = NPC // W2_TILE       # 10
POS_SCALE = 1000.0

_cache = {}


def _gumbel_const():
    if "G" not in _cache:
        import jax
        import jax.numpy as jnp
        g = jax.random.gumbel(jax.random.key(42), (SAMPLE, NODE, SEG),
                              jnp.float32)
        _cache["G"] = np.asarray(jax.device_get(g))
    return _cache["G"]


def _bcast(ap, dim, count):
    """Insert a step-0 (broadcast) dim into an AP after position `dim`."""
    new = list(ap.ap)
    new.insert(dim, [0, count])
    return bass.AP(ap.tensor, ap.offset, new)


def _build_graph():
    nc = bacc.Bacc("TRN2", target_bir_lowering=False, debug=False,
                   num_devices=N_CORES)

    xw = nc.dram_tensor("xw", [128, NKF], F32, kind="ExternalInput")
    w1 = nc.dram_tensor("w1", [128, NKF, HIDDEN], F32, kind="ExternalInput")
    b1 = nc.dram_tensor("b1", [HIDDEN, 1], F32, kind="ExternalInput")
    w2 = nc.dram_tensor("w2", [HIDDEN, NPC], F32, kind="ExternalInput")
    b2t = nc.dram_tensor("b2t", [NTILES, TILE_P, SEG], F32, kind="ExternalInput")
    iota1000 = nc.dram_tensor("iota1000", [TILE_P, SEG], F32, kind="ExternalInput")
    g = nc.dram_tensor("g", [NTILES, TILE_P, SAMPLE, SEG], F32, kind="ExternalInput")

    scores_out = nc.dram_tensor("scores_out", [NPC], F32, kind="ExternalOutput")
    v_out = nc.dram_tensor("v_out", [NTILES, TILE_P, SAMPLE], F32, kind="ExternalOutput")

    sc_view = scores_out.ap().rearrange("(t p j) -> t p j", p=TILE_P, j=SEG)


    with tile.TileContext(nc) as tc:
        with (
            tc.tile_pool(name="const", bufs=1) as const,
            tc.tile_pool(name="w2p", bufs=4) as w2p,
            tc.tile_pool(name="rawcb", bufs=2) as rawcb,
            tc.tile_pool(name="gp", bufs=4) as gp,
            tc.tile_pool(name="small", bufs=4) as small,
            tc.tile_pool(name="sampw", bufs=3) as sampw,
            tc.tile_pool(name="eqp", bufs=3) as eqp,
            tc.tile_pool(name="mxp", bufs=4) as mxp,
            tc.tile_pool(name="vall", bufs=1) as vall,
            tc.tile_pool(name="w1p", bufs=10) as w1p,
            tc.tile_pool(name="ph", bufs=1, space="PSUM") as ph,
            tc.tile_pool(name="pch", bufs=6, space="PSUM") as pch,
        ):
            # ---- constants ----
            xw_sb = const.tile([128, NKF], F32)
            nc.sync.dma_start(out=xw_sb[:], in_=xw[:, :])
            b1_sb = const.tile([HIDDEN, 1], F32)
            nc.sync.dma_start(out=b1_sb[:], in_=b1[:, :])
            iota_sb = const.tile([TILE_P, SEG], F32)
            nc.sync.dma_start(out=iota_sb[:], in_=iota1000[:, :])
            b2_sb = []
            for t in range(NTILES):
                b2_t = const.tile([TILE_P, SEG], F32, tag=f"b2_{t}")
                nc.sync.dma_start(out=b2_t[:], in_=b2t[t])
                b2_sb.append(b2_t)
            # ---- matmul1 (replicated W1, x stationary / W1 moving) ----
            psum_hT = ph.tile([1, HIDDEN], F32)
            for gr in range(N_W1_GRP):
                w1_sb = w1p.tile([128, W1_GRP, HIDDEN], F32)
                nc.sync.dma_start(out=w1_sb[:],
                                  in_=w1[:, gr * W1_GRP:(gr + 1) * W1_GRP, :])
                for kk in range(W1_GRP):
                    k = gr * W1_GRP + kk
                    nc.tensor.matmul(
                        psum_hT[:], lhsT=xw_sb[:, k:k + 1], rhs=w1_sb[:, kk, :],
                        start=(k == 0), stop=(k == NKF - 1))
            zrow = const.tile([1, HIDDEN], F32, tag="zrow")
            nc.scalar.activation(zrow[:], psum_hT[:], ACTF.Copy)
            zcol = const.tile([HIDDEN, 1], F32, tag="zcol")
            nc.scalar.dma_start(out=zcol[:], in_=zrow[:])
            h_sb = const.tile([HIDDEN, 1], F32, tag="h")
            nc.scalar.activation(h_sb[:], zcol[:], ACTF.Tanh, bias=b1_sb[:])

            # ---- matmul2 + logits + sampling, interleaved emission ----
            def emit_mm2(t):
                raw_t = const.tile([TILE_P, SEG], F32, tag=f"raw{t}")
                for wt in range(N_W2_TILE // NTILES * t,
                                N_W2_TILE // NTILES * (t + 1)):
                    w2_sb = w2p.tile([HIDDEN, W2_TILE], F32)
                    half = W2_TILE // 2
                    nc.sync.dma_start(
                        out=w2_sb[:, :half],
                        in_=w2[:, wt * W2_TILE:wt * W2_TILE + half])
                    nc.sync.dma_start(
                        out=w2_sb[:, half:],
                        in_=w2[:, wt * W2_TILE + half:(wt + 1) * W2_TILE])
                    rawcb_t = rawcb.tile([1, W2_TILE], F32, tag="rawcb")
                    for cc in range(W2_TILE // MM2):
                        psum_c = pch.tile([1, MM2], F32)
                        nc.tensor.matmul(
                            psum_c[:], lhsT=h_sb[:],
                            rhs=w2_sb[:, cc * MM2:(cc + 1) * MM2],
                            start=True, stop=True)
                        nc.scalar.activation(
                            rawcb_t[0:1, cc * MM2:(cc + 1) * MM2], psum_c[:],
                            ACTF.Copy)
                    wrow = (wt % (N_W2_TILE // NTILES)) * (W2_TILE // SEG)
                    nc.scalar.dma_start(
                        out=raw_t[wrow:wrow + W2_TILE // SEG, :], in_=rawcb_t[:])
                return raw_t

            def emit_pipe(t, raw_t):
                z = small.tile([TILE_P, SEG], F32, tag="z")
                nc.vector.tensor_add(out=z[:], in0=raw_t[:], in1=b2_sb[t][:])
                th = small.tile([TILE_P, SEG], F32, tag="th")
                nc.scalar.activation(th[:], z[:], ACTF.Tanh)
                logits = const.tile([TILE_P, SEG], F32, tag=f"logits{t}")
                nc.vector.tensor_scalar_mul(logits[:], th[:], C_SCALE)
                nc.scalar.dma_start(out=sc_view[t], in_=logits[:])
                mt = small.tile([TILE_P, 1], F32, tag="mt")
                nc.vector.reduce_max(out=mt[:], in_=logits[:], axis=AX.X)
                negm = small.tile([TILE_P, 1], F32, tag="negm")
                nc.vector.tensor_scalar_mul(negm[:], mt[:], -1.0)
                e = small.tile([TILE_P, SEG], F32, tag="e")
                ssum = small.tile([TILE_P, 1], F32, tag="ssum")
                nc.scalar.activation(e[:], logits[:], ACTF.Exp, bias=negm[:],
                                     accum_out=ssum[:])
                lse = small.tile([TILE_P, 1], F32, tag="lse")
                nc.scalar.activation(lse[:], ssum[:], ACTF.Ln)
                sub = small.tile([TILE_P, 1], F32, tag="sub")
                nc.scalar.activation(sub[:], lse[:], ACTF.Identity,
                                     bias=negm[:], scale=-1.0)
                comb = const.tile([TILE_P, SEG], F32, tag=f"comb{t}")
                nc.vector.scalar_tensor_tensor(
                    out=comb[:], in0=iota_sb[:], scalar=sub[:], in1=logits[:],
                    op0=ALU.add, op1=ALU.add)
                return logits, comb

            def emit_g_prefetch(t):
                tiles = []
                for b in range(NBLK):
                    g_sb = gp.tile([TILE_P, SBLK, SEG], F32, tag="g")
                    gh = SBLK // 2
                    nc.sync.dma_start(out=g_sb[:, :gh, :],
                                      in_=g[t, :, b * SBLK:b * SBLK + gh, :])
                    nc.sync.dma_start(out=g_sb[:, gh:, :],
                                      in_=g[t, :, b * SBLK + gh:(b + 1) * SBLK, :])
                    tiles.append(g_sb)
                return tiles

            def emit_sampling(t, logits, comb, g_tiles):
                v_sb = vall.tile([TILE_P, SAMPLE], F32, tag=f"v{t}")
                for b in range(NBLK):
                    g_sb = g_tiles[b]
                    tmp = sampw.tile([TILE_P, SBLK, SEG], F32, tag="tmp")
                    l_b = _bcast(logits[:], 1, SBLK)
                    nc.gpsimd.tensor_tensor(out=tmp[:], in0=g_sb[:], in1=l_b,
                                            op=ALU.add)
                    negmx = mxp.tile([TILE_P, SBLK], F32, tag="negmx")
                    nc.vector.tensor_reduce(out=negmx[:], in_=tmp[:], axis=AX.X,
                                            op=ALU.max, negate=True)
                    for sl in range(SBLK):
                        nc.scalar.activation(tmp[:, sl, :], tmp[:, sl, :],
                                             ACTF.Sign,
                                             bias=negmx[:, sl:sl + 1])
                    c_b = _bcast(comb[:], 1, SBLK)
                    nc.vector.scalar_tensor_tensor(
                        out=tmp[:], in0=tmp[:], scalar=0.0, in1=c_b,
                        op0=ALU.is_ge, op1=ALU.mult)
                    nc.vector.reduce_sum(out=v_sb[:, b * SBLK:(b + 1) * SBLK],
                                         in_=tmp[:], axis=AX.X)
                nc.scalar.dma_start(out=v_out[t], in_=v_sb[:])

            raw0 = emit_mm2(0)
            logits0, comb0 = emit_pipe(0, raw0)
            g0 = emit_g_prefetch(0)
            raw1 = emit_mm2(1)
            emit_sampling(0, logits0, comb0, g0)
            logits1, comb1 = emit_pipe(1, raw1)
            g1 = emit_g_prefetch(1)
            emit_sampling(1, logits1, comb1, g1)

    nc.compile()
    return nc


def _get_graph():
    if "nc" not in _cache:
        _cache["nc"] = _build_graph()
    return _cache["nc"]


def kernel(inputs, W1, b1, W2, b2, num_service):
    inputs = np.ascontiguousarray(np.asarray(inputs, dtype=np.float32))
    W1 = np.asarray(W1, dtype=np.float32)
    b1v = np.asarray(b1, dtype=np.float32).reshape(HIDDEN, 1)
    W2 = np.asarray(W2, dtype=np.float32)
    b2v = np.asarray(b2, dtype=np.float32)

    G = _gumbel_const()
    if "Gn" not in _cache:
        _cache["Gn"] = np.ascontiguousarray(np.transpose(G, (1, 0, 2)))
    Gn = _cache["Gn"]

    iota_host = np.tile((POS_SCALE * np.arange(SEG, dtype=np.float32))[None, :],
                        (TILE_P, 1))
    xw_host = np.ascontiguousarray(inputs.reshape(NKF, 128).T)   # [128, NKF]
    # W1 [16384,128] -> [128 lane, 128 chunk, 128 col], contiguous per lane
    w1t_host = np.ascontiguousarray(
        W1.reshape(NKF, 128, HIDDEN).transpose(1, 0, 2))

    in_maps = []
    for c in range(N_CORES):
        in_maps.append({
            "xw": xw_host,
            "w1": w1t_host,
            "b1": b1v,
            "w2": np.ascontiguousarray(W2[:, c * NPC:(c + 1) * NPC]),
            "b2t": np.ascontiguousarray(
                b2v[c * NPC:(c + 1) * NPC].reshape(NTILES, TILE_P, SEG)),
            "iota1000": iota_host,
            "g": np.ascontiguousarray(
                Gn[c * SEGS_PC:(c + 1) * SEGS_PC].reshape(NTILES, TILE_P, SAMPLE, SEG)),
        })

    nc = _get_graph()
    trace = bool(int(os.environ.get("KERNEL_TRACE", "0")))
    res = run_bass_kernel_spmd(nc, in_maps, core_ids=list(range(N_CORES)),
                               trace=trace)
    if trace and res.exec_time_ns is not None:
        print(f"HW exec time: {res.exec_time_ns} ns")
        _cache["exec_time_ns"] = res.exec_time_ns
        _cache["results"] = res

    scores = np.concatenate([np.asarray(r["scores_out"]).ravel()
                             for r in res.results])
    v = np.concatenate(
        [np.asarray(r["v_out"]).reshape(SEGS_PC, SAMPLE) for r in res.results],
        axis=0)                                    # [2000, 128]
    v = v.T                                        # [sample, node]
    pos = np.round(v / POS_SCALE).astype(np.int32)  # exact: |lsm| < 25 << 500
    lsm_p = v.astype(np.float64) - POS_SCALE * pos
    log_softmax = np.sum(lsm_p, axis=1).astype(np.float32)
    return pos, log_softmax, scores


# revision 30
# speedup vs baseline: 1.0673x; 1.0673x over previous
"""Trainium2 Bass kernel for nn_Actor (segmented categorical sampling head).

Math (matches reference):
  h      = tanh(inputs @ W1 + b1)                    [1,128]
  scores = 10*tanh(h @ W2 + b2)                      [200000]
  logits = scores.reshape(2000, 100)
  G      = jax.random.gumbel(key(42), (128,2000,100), f32)   (input-independent)
  positions[s,n] = argmax_j(logits[n,j] + G[s,n,j])
  log_softmax[s] = sum_n log_softmax(logits)[n, positions[s,n]]

Sharding: 8-way tensor parallel over all_node_num (25000 scores = 250
segments per core). W1/inputs are sharded over input_dim with an on-device
AllReduce of the partial h. Each core computes its local scores + samples
its local segments; host concatenates positions/scores and sums log-probs.

Device encoding trick: per (sample, segment) the kernel emits
  v = lsm[pos] + 1000*pos
via blocked DVE/GpSimd ops: tmp = G+logits (broadcast AP over samples),
mx = max_j tmp, eq = (tmp == mx) (broadcast over j), prod = eq*comb
(comb = lsm + 1000*j), v = sum_j prod. Host decodes pos = round(v/1000)
exactly and lsm[pos] = v - 1000*pos.

G must be generated on the default jax backend — the same backend the
reference's jax.random.categorical runs on (the env's default PRNG impl
"rbg" is backend-dependent).
"""

import os
import numpy as np

import concourse.bass as bass
import concourse.bacc as bacc
import concourse.tile as tile
from concourse import mybir
from concourse.bass_utils import run_bass_kernel_spmd

F32 = mybir.dt.float32
AX = mybir.AxisListType
ALU = mybir.AluOpType
ACTF = mybir.ActivationFunctionType

N_CORES = 8
INPUT_DIM = 16384
HIDDEN = 128
ALL_NODE = 200000
NODE = 2000
SEG = 100
SAMPLE = 128
C_SCALE = 10.0

NPC = ALL_NODE // N_CORES        # 25000 scores per core
SEGS_PC = NODE // N_CORES        # 250 segments per core
TILE_P = 125                     # segment partitions per tile
NTILES = SEGS_PC // TILE_P       # 2
SBLK = 32                        # samples per G block
NBLK = SAMPLE // SBLK            # 4
KPC = INPUT_DIM // N_CORES       # 2048 input rows per core
NK = KPC // 128                  # 16 k-chunks per core
NKF = INPUT_DIM // 128           # 128 k-chunks (replicated W1)
W1_GRP = 8                       # chunks per W1 DMA group
N_W1_GRP = NKF // W1_GRP         # 16
MM2 = 500                        # w2 columns per matmul (= 5 segments)
NCH = NPC // MM2                 # 50 chunks; 25 per tile
W2_TILE = 2500                   # w2 columns per DMA tile
N_W2_TILE = NPC // W2_TILE       # 10
POS_SCALE = 1000.0

_cache = {}


def _gumbel_const():
    if "G" not in _cache:
        import jax
        import jax.numpy as jnp
        g = jax.random.gumbel(jax.random.key(42), (SAMPLE, NODE, SEG),
                              jnp.float32)
        _cache["G"] = np.asarray(jax.device_get(g))
    return _cache["G"]


def _bcast(ap, dim, count):
    """Insert a step-0 (broadcast) dim into an AP after position `dim`."""
    new = list(ap.ap)
    new.insert(dim, [0, count])
    return bass.AP(ap.tensor, ap.offset, new)


def _build_graph():
    nc = bacc.Bacc("TRN2", target_bir_lowering=False, debug=False,
                   num_devices=N_CORES)

    xw = nc.dram_tensor("xw", [128, NKF], F32, kind="ExternalInput")
    w1 = nc.dram_tensor("w1", [128, NKF, HIDDEN], F32, kind="ExternalInput")
    b1 = nc.dram_tensor("b1", [HIDDEN, 1], F32, kind="ExternalInput")
    w2 = nc.dram_tensor("w2", [HIDDEN, NPC], F32, kind="ExternalInput")
    b2t = nc.dram_tensor("b2t", [NTILES, TILE_P, SEG], F32, kind="ExternalInput")
    iota1000 = nc.dram_tensor("iota1000", [TILE_P, SEG], F32, kind="ExternalInput")
    g = nc.dram_tensor("g", [NTILES, TILE_P, SAMPLE, SEG], F32, kind="ExternalInput")

    scores_out = nc.dram_tensor("scores_out", [NPC], F32, kind="ExternalOutput")
    v_out = nc.dram_tensor("v_out", [NTILES, TILE_P, SAMPLE], F32, kind="ExternalOutput")

    sc_view = scores_out.ap().rearrange("(t p j) -> t p j", p=TILE_P, j=SEG)


    with tile.TileContext(nc) as tc:
        with (
            tc.tile_pool(name="const", bufs=1) as const,
            tc.tile_pool(name="w2p", bufs=4) as w2p,
            tc.tile_pool(name="rawcb", bufs=2) as rawcb,
            tc.tile_pool(name="gp", bufs=3) as gp,
            tc.tile_pool(name="small", bufs=4) as small,
            tc.tile_pool(name="sampw", bufs=4) as sampw,
            tc.tile_pool(name="eqp", bufs=3) as eqp,
            tc.tile_pool(name="mxp", bufs=4) as mxp,
            tc.tile_pool(name="vall", bufs=1) as vall,
            tc.tile_pool(name="w1p", bufs=10) as w1p,
            tc.tile_pool(name="ph", bufs=1, space="PSUM") as ph,
            tc.tile_pool(name="pch", bufs=6, space="PSUM") as pch,
        ):
            # ---- constants ----
            xw_sb = const.tile([128, NKF], F32)
            nc.sync.dma_start(out=xw_sb[:], in_=xw[:, :])
            b1_sb = const.tile([HIDDEN, 1], F32)
            nc.sync.dma_start(out=b1_sb[:], in_=b1[:, :])
            iota_sb = const.tile([TILE_P, SEG], F32)
            nc.sync.dma_start(out=iota_sb[:], in_=iota1000[:, :])
            b2_sb = []
            for t in range(NTILES):
                b2_t = const.tile([TILE_P, SEG], F32, tag=f"b2_{t}")
                nc.sync.dma_start(out=b2_t[:], in_=b2t[t])
                b2_sb.append(b2_t)
            # ---- matmul1 (replicated W1, x stationary / W1 moving) ----
            psum_hT = ph.tile([1, HIDDEN], F32)
            for gr in range(N_W1_GRP):
                w1_sb = w1p.tile([128, W1_GRP, HIDDEN], F32)
                nc.sync.dma_start(out=w1_sb[:],
                                  in_=w1[:, gr * W1_GRP:(gr + 1) * W1_GRP, :])
                for kk in range(W1_GRP):
                    k = gr * W1_GRP + kk
                    nc.tensor.matmul(
                        psum_hT[:], lhsT=xw_sb[:, k:k + 1], rhs=w1_sb[:, kk, :],
                        start=(k == 0), stop=(k == NKF - 1))
            zrow = const.tile([1, HIDDEN], F32, tag="zrow")
            nc.scalar.activation(zrow[:], psum_hT[:], ACTF.Copy)
            zcol = const.tile([HIDDEN, 1], F32, tag="zcol")
            nc.scalar.dma_start(out=zcol[:], in_=zrow[:])
            h_sb = const.tile([HIDDEN, 1], F32, tag="h")
            nc.scalar.activation(h_sb[:], zcol[:], ACTF.Tanh, bias=b1_sb[:])

            # ---- matmul2 + logits + sampling, interleaved emission ----
            def emit_mm2(t):
                raw_t = const.tile([TILE_P, SEG], F32, tag=f"raw{t}")
                for wt in range(N_W2_TILE // NTILES * t,
                                N_W2_TILE // NTILES * (t + 1)):
                    w2_sb = w2p.tile([HIDDEN, W2_TILE], F32)
                    q = W2_TILE // 4
                    for qq in range(4):
                        nc.sync.dma_start(
                            out=w2_sb[:, qq * q:(qq + 1) * q],
                            in_=w2[:, wt * W2_TILE + qq * q:wt * W2_TILE + (qq + 1) * q])
                    rawcb_t = rawcb.tile([1, W2_TILE], F32, tag="rawcb")
                    for cc in range(W2_TILE // MM2):
                        psum_c = pch.tile([1, MM2], F32)
                        nc.tensor.matmul(
                            psum_c[:], lhsT=h_sb[:],
                            rhs=w2_sb[:, cc * MM2:(cc + 1) * MM2],
                            start=True, stop=True)
                        nc.scalar.activation(
                            rawcb_t[0:1, cc * MM2:(cc + 1) * MM2], psum_c[:],
                            ACTF.Copy)
                    wrow = (wt % (N_W2_TILE // NTILES)) * (W2_TILE // SEG)
                    nc.scalar.dma_start(
                        out=raw_t[wrow:wrow + W2_TILE // SEG, :], in_=rawcb_t[:])
                return raw_t

            def emit_pipe(t, raw_t):
                z = small.tile([TILE_P, SEG], F32, tag="z")
                nc.vector.tensor_add(out=z[:], in0=raw_t[:], in1=b2_sb[t][:])
                th = small.tile([TILE_P, SEG], F32, tag="th")
                nc.scalar.activation(th[:], z[:], ACTF.Tanh)
                logits = const.tile([TILE_P, SEG], F32, tag=f"logits{t}")
                nc.vector.tensor_scalar_mul(logits[:], th[:], C_SCALE)
                nc.scalar.dma_start(out=sc_view[t], in_=logits[:])
                mt = small.tile([TILE_P, 1], F32, tag="mt")
                nc.vector.reduce_max(out=mt[:], in_=logits[:], axis=AX.X)
                negm = small.tile([TILE_P, 1], F32, tag="negm")
                nc.vector.tensor_scalar_mul(negm[:], mt[:], -1.0)
                e = small.tile([TILE_P, SEG], F32, tag="e")
                ssum = small.tile([TILE_P, 1], F32, tag="ssum")
                nc.scalar.activation(e[:], logits[:], ACTF.Exp, bias=negm[:],
                                     accum_out=ssum[:])
                lse = small.tile([TILE_P, 1], F32, tag="lse")
                nc.scalar.activation(lse[:], ssum[:], ACTF.Ln)
                sub = small.tile([TILE_P, 1], F32, tag="sub")
                nc.scalar.activation(sub[:], lse[:], ACTF.Identity,
                                     bias=negm[:], scale=-1.0)
                comb = const.tile([TILE_P, SEG], F32, tag=f"comb{t}")
                nc.vector.scalar_tensor_tensor(
                    out=comb[:], in0=iota_sb[:], scalar=sub[:], in1=logits[:],
                    op0=ALU.add, op1=ALU.add)
                return logits, comb

            def emit_g_prefetch(t):
                tiles = []
                for b in range(NBLK):
                    g_sb = gp.tile([TILE_P, SBLK, SEG], F32, tag="g")
                    gq = SBLK // 4
                    for qq in range(4):
                        nc.sync.dma_start(
                            out=g_sb[:, qq * gq:(qq + 1) * gq, :],
                            in_=g[t, :, b * SBLK + qq * gq:b * SBLK + (qq + 1) * gq, :])
                    tiles.append(g_sb)
                return tiles

            def emit_sampling(t, logits, comb, g_tiles):
                v_sb = vall.tile([TILE_P, SAMPLE], F32, tag=f"v{t}")
                for b in range(NBLK):
                    g_sb = g_tiles[b]
                    tmp = sampw.tile([TILE_P, SBLK, SEG], F32, tag="tmp")
                    l_b = _bcast(logits[:], 1, SBLK)
                    nc.gpsimd.tensor_tensor(out=tmp[:], in0=g_sb[:], in1=l_b,
                                            op=ALU.add)
                    negmx = mxp.tile([TILE_P, SBLK], F32, tag="negmx")
                    nc.vector.tensor_reduce(out=negmx[:], in_=tmp[:], axis=AX.X,
                                            op=ALU.max, negate=True)
                    for sl in range(SBLK):
                        nc.scalar.activation(tmp[:, sl, :], tmp[:, sl, :],
                                             ACTF.Sign,
                                             bias=negmx[:, sl:sl + 1])
                    c_b = _bcast(comb[:], 1, SBLK)
                    nc.vector.scalar_tensor_tensor(
                        out=tmp[:], in0=tmp[:], scalar=0.0, in1=c_b,
                        op0=ALU.is_ge, op1=ALU.mult)
                    nc.vector.reduce_sum(out=v_sb[:, b * SBLK:(b + 1) * SBLK],
                                         in_=tmp[:], axis=AX.X)
                nc.scalar.dma_start(out=v_out[t], in_=v_sb[:])

            raw0 = emit_mm2(0)
            logits0, comb0 = emit_pipe(0, raw0)
            g0 = emit_g_prefetch(0)
            raw1 = emit_mm2(1)
            emit_sampling(0, logits0, comb0, g0)
            logits1, comb1 = emit_pipe(1, raw1)
            g1 = emit_g_prefetch(1)
            emit_sampling(1, logits1, comb1, g1)

    nc.compile()
    return nc


def _get_graph():
    if "nc" not in _cache:
        _cache["nc"] = _build_graph()
    return _cache["nc"]


def kernel(inputs, W1, b1, W2, b2, num_service):
    inputs = np.ascontiguousarray(np.asarray(inputs, dtype=np.float32))
    W1 = np.asarray(W1, dtype=np.float32)
    b1v = np.asarray(b1, dtype=np.float32).reshape(HIDDEN, 1)
    W2 = np.asarray(W2, dtype=np.float32)
    b2v = np.asarray(b2, dtype=np.float32)

    G = _gumbel_const()
    if "Gn" not in _cache:
        _cache["Gn"] = np.ascontiguousarray(np.transpose(G, (1, 0, 2)))
    Gn = _cache["Gn"]

    iota_host = np.tile((POS_SCALE * np.arange(SEG, dtype=np.float32))[None, :],
                        (TILE_P, 1))
    xw_host = np.ascontiguousarray(inputs.reshape(NKF, 128).T)   # [128, NKF]
    # W1 [16384,128] -> [128 lane, 128 chunk, 128 col], contiguous per lane
    w1t_host = np.ascontiguousarray(
        W1.reshape(NKF, 128, HIDDEN).transpose(1, 0, 2))

    in_maps = []
    for c in range(N_CORES):
        in_maps.append({
            "xw": xw_host,
            "w1": w1t_host,
            "b1": b1v,
            "w2": np.ascontiguousarray(W2[:, c * NPC:(c + 1) * NPC]),
            "b2t": np.ascontiguousarray(
                b2v[c * NPC:(c + 1) * NPC].reshape(NTILES, TILE_P, SEG)),
            "iota1000": iota_host,
            "g": np.ascontiguousarray(
                Gn[c * SEGS_PC:(c + 1) * SEGS_PC].reshape(NTILES, TILE_P, SAMPLE, SEG)),
        })

    nc = _get_graph()
    trace = bool(int(os.environ.get("KERNEL_TRACE", "0")))
    res = run_bass_kernel_spmd(nc, in_maps, core_ids=list(range(N_CORES)),
                               trace=trace)
    if trace and res.exec_time_ns is not None:
        print(f"HW exec time: {res.exec_time_ns} ns")
        _cache["exec_time_ns"] = res.exec_time_ns
        _cache["results"] = res

    scores = np.concatenate([np.asarray(r["scores_out"]).ravel()
                             for r in res.results])
    v = np.concatenate(
        [np.asarray(r["v_out"]).reshape(SEGS_PC, SAMPLE) for r in res.results],
        axis=0)                                    # [2000, 128]
    v = v.T                                        # [sample, node]
    pos = np.round(v / POS_SCALE).astype(np.int32)  # exact: |lsm| < 25 << 500
    lsm_p = v.astype(np.float64) - POS_SCALE * pos
    log_softmax = np.sum(lsm_p, axis=1).astype(np.float32)
    return pos, log_softmax, scores


# revision 31
# speedup vs baseline: 1.1308x; 1.0595x over previous
"""Trainium2 Bass kernel for nn_Actor (segmented categorical sampling head).

Math (matches reference):
  h      = tanh(inputs @ W1 + b1)                    [1,128]
  scores = 10*tanh(h @ W2 + b2)                      [200000]
  logits = scores.reshape(2000, 100)
  G      = jax.random.gumbel(key(42), (128,2000,100), f32)   (input-independent)
  positions[s,n] = argmax_j(logits[n,j] + G[s,n,j])
  log_softmax[s] = sum_n log_softmax(logits)[n, positions[s,n]]

Sharding: 8-way tensor parallel over all_node_num (25000 scores = 250
segments per core). W1/inputs are sharded over input_dim with an on-device
AllReduce of the partial h. Each core computes its local scores + samples
its local segments; host concatenates positions/scores and sums log-probs.

Device encoding trick: per (sample, segment) the kernel emits
  v = lsm[pos] + 1000*pos
via blocked DVE/GpSimd ops: tmp = G+logits (broadcast AP over samples),
mx = max_j tmp, eq = (tmp == mx) (broadcast over j), prod = eq*comb
(comb = lsm + 1000*j), v = sum_j prod. Host decodes pos = round(v/1000)
exactly and lsm[pos] = v - 1000*pos.

G must be generated on the default jax backend — the same backend the
reference's jax.random.categorical runs on (the env's default PRNG impl
"rbg" is backend-dependent).
"""

import os
import numpy as np

import concourse.bass as bass
import concourse.bacc as bacc
import concourse.tile as tile
from concourse import mybir
from concourse.bass_utils import run_bass_kernel_spmd

F32 = mybir.dt.float32
AX = mybir.AxisListType
ALU = mybir.AluOpType
ACTF = mybir.ActivationFunctionType

N_CORES = 8
INPUT_DIM = 16384
HIDDEN = 128
ALL_NODE = 200000
NODE = 2000
SEG = 100
SAMPLE = 128
C_SCALE = 10.0

NPC = ALL_NODE // N_CORES        # 25000 scores per core
SEGS_PC = NODE // N_CORES        # 250 segments per core
TILE_P = 125                     # segment partitions per tile
NTILES = SEGS_PC // TILE_P       # 2
SBLK = 16                        # samples per G block
NBLK = SAMPLE // SBLK            # 8
KPC = INPUT_DIM // N_CORES       # 2048 input rows per core
NK = KPC // 128                  # 16 k-chunks per core
NKF = INPUT_DIM // 128           # 128 k-chunks (replicated W1)
W1_GRP = 8                       # chunks per W1 DMA group
N_W1_GRP = NKF // W1_GRP         # 16
MM2 = 500                        # w2 columns per matmul (= 5 segments)
NCH = NPC // MM2                 # 50 chunks; 25 per tile
W2_TILE = 2500                   # w2 columns per DMA tile
N_W2_TILE = NPC // W2_TILE       # 10
POS_SCALE = 1000.0

_cache = {}


def _gumbel_const():
    if "G" not in _cache:
        import jax
        import jax.numpy as jnp
        g = jax.random.gumbel(jax.random.key(42), (SAMPLE, NODE, SEG),
                              jnp.float32)
        _cache["G"] = np.asarray(jax.device_get(g))
    return _cache["G"]


def _bcast(ap, dim, count):
    """Insert a step-0 (broadcast) dim into an AP after position `dim`."""
    new = list(ap.ap)
    new.insert(dim, [0, count])
    return bass.AP(ap.tensor, ap.offset, new)


def _build_graph():
    nc = bacc.Bacc("TRN2", target_bir_lowering=False, debug=False,
                   num_devices=N_CORES)

    xw = nc.dram_tensor("xw", [128, NKF], F32, kind="ExternalInput")
    w1 = nc.dram_tensor("w1", [128, NKF, HIDDEN], F32, kind="ExternalInput")
    b1 = nc.dram_tensor("b1", [HIDDEN, 1], F32, kind="ExternalInput")
    w2 = nc.dram_tensor("w2", [HIDDEN, NPC], F32, kind="ExternalInput")
    b2t = nc.dram_tensor("b2t", [NTILES, TILE_P, SEG], F32, kind="ExternalInput")
    iota1000 = nc.dram_tensor("iota1000", [TILE_P, SEG], F32, kind="ExternalInput")
    g = nc.dram_tensor("g", [NTILES, TILE_P, SAMPLE, SEG], F32, kind="ExternalInput")

    scores_out = nc.dram_tensor("scores_out", [NPC], F32, kind="ExternalOutput")
    v_out = nc.dram_tensor("v_out", [NTILES, TILE_P, SAMPLE], F32, kind="ExternalOutput")

    sc_view = scores_out.ap().rearrange("(t p j) -> t p j", p=TILE_P, j=SEG)


    with tile.TileContext(nc) as tc:
        with (
            tc.tile_pool(name="const", bufs=1) as const,
            tc.tile_pool(name="w2p", bufs=4) as w2p,
            tc.tile_pool(name="rawcb", bufs=2) as rawcb,
            tc.tile_pool(name="gp", bufs=6) as gp,
            tc.tile_pool(name="small", bufs=4) as small,
            tc.tile_pool(name="sampw", bufs=6) as sampw,
            tc.tile_pool(name="eqp", bufs=3) as eqp,
            tc.tile_pool(name="mxp", bufs=4) as mxp,
            tc.tile_pool(name="vall", bufs=1) as vall,
            tc.tile_pool(name="w1p", bufs=10) as w1p,
            tc.tile_pool(name="ph", bufs=1, space="PSUM") as ph,
            tc.tile_pool(name="pch", bufs=6, space="PSUM") as pch,
        ):
            # ---- constants ----
            xw_sb = const.tile([128, NKF], F32)
            nc.sync.dma_start(out=xw_sb[:], in_=xw[:, :])
            b1_sb = const.tile([HIDDEN, 1], F32)
            nc.sync.dma_start(out=b1_sb[:], in_=b1[:, :])
            iota_sb = const.tile([TILE_P, SEG], F32)
            nc.sync.dma_start(out=iota_sb[:], in_=iota1000[:, :])
            b2_sb = []
            for t in range(NTILES):
                b2_t = const.tile([TILE_P, SEG], F32, tag=f"b2_{t}")
                nc.sync.dma_start(out=b2_t[:], in_=b2t[t])
                b2_sb.append(b2_t)
            # ---- matmul1 (replicated W1, x stationary / W1 moving) ----
            psum_hT = ph.tile([1, HIDDEN], F32)
            for gr in range(N_W1_GRP):
                w1_sb = w1p.tile([128, W1_GRP, HIDDEN], F32)
                nc.sync.dma_start(out=w1_sb[:],
                                  in_=w1[:, gr * W1_GRP:(gr + 1) * W1_GRP, :])
                for kk in range(W1_GRP):
                    k = gr * W1_GRP + kk
                    nc.tensor.matmul(
                        psum_hT[:], lhsT=xw_sb[:, k:k + 1], rhs=w1_sb[:, kk, :],
                        start=(k == 0), stop=(k == NKF - 1))
            zrow = const.tile([1, HIDDEN], F32, tag="zrow")
            nc.scalar.activation(zrow[:], psum_hT[:], ACTF.Copy)
            zcol = const.tile([HIDDEN, 1], F32, tag="zcol")
            nc.scalar.dma_start(out=zcol[:], in_=zrow[:])
            h_sb = const.tile([HIDDEN, 1], F32, tag="h")
            nc.scalar.activation(h_sb[:], zcol[:], ACTF.Tanh, bias=b1_sb[:])

            # ---- matmul2 + logits + sampling, interleaved emission ----
            def emit_mm2(t):
                raw_t = const.tile([TILE_P, SEG], F32, tag=f"raw{t}")
                for wt in range(N_W2_TILE // NTILES * t,
                                N_W2_TILE // NTILES * (t + 1)):
                    w2_sb = w2p.tile([HIDDEN, W2_TILE], F32)
                    q = W2_TILE // 4
                    for qq in range(4):
                        nc.sync.dma_start(
                            out=w2_sb[:, qq * q:(qq + 1) * q],
                            in_=w2[:, wt * W2_TILE + qq * q:wt * W2_TILE + (qq + 1) * q])
                    rawcb_t = rawcb.tile([1, W2_TILE], F32, tag="rawcb")
                    for cc in range(W2_TILE // MM2):
                        psum_c = pch.tile([1, MM2], F32)
                        nc.tensor.matmul(
                            psum_c[:], lhsT=h_sb[:],
                            rhs=w2_sb[:, cc * MM2:(cc + 1) * MM2],
                            start=True, stop=True)
                        nc.scalar.activation(
                            rawcb_t[0:1, cc * MM2:(cc + 1) * MM2], psum_c[:],
                            ACTF.Copy)
                    wrow = (wt % (N_W2_TILE // NTILES)) * (W2_TILE // SEG)
                    nc.scalar.dma_start(
                        out=raw_t[wrow:wrow + W2_TILE // SEG, :], in_=rawcb_t[:])
                return raw_t

            def emit_pipe(t, raw_t):
                z = small.tile([TILE_P, SEG], F32, tag="z")
                nc.vector.tensor_add(out=z[:], in0=raw_t[:], in1=b2_sb[t][:])
                th = small.tile([TILE_P, SEG], F32, tag="th")
                nc.scalar.activation(th[:], z[:], ACTF.Tanh)
                logits = const.tile([TILE_P, SEG], F32, tag=f"logits{t}")
                nc.vector.tensor_scalar_mul(logits[:], th[:], C_SCALE)
                nc.scalar.dma_start(out=sc_view[t], in_=logits[:])
                mt = small.tile([TILE_P, 1], F32, tag="mt")
                nc.vector.reduce_max(out=mt[:], in_=logits[:], axis=AX.X)
                negm = small.tile([TILE_P, 1], F32, tag="negm")
                nc.vector.tensor_scalar_mul(negm[:], mt[:], -1.0)
                e = small.tile([TILE_P, SEG], F32, tag="e")
                ssum = small.tile([TILE_P, 1], F32, tag="ssum")
                nc.scalar.activation(e[:], logits[:], ACTF.Exp, bias=negm[:],
                                     accum_out=ssum[:])
                lse = small.tile([TILE_P, 1], F32, tag="lse")
                nc.scalar.activation(lse[:], ssum[:], ACTF.Ln)
                sub = small.tile([TILE_P, 1], F32, tag="sub")
                nc.scalar.activation(sub[:], lse[:], ACTF.Identity,
                                     bias=negm[:], scale=-1.0)
                comb = const.tile([TILE_P, SEG], F32, tag=f"comb{t}")
                nc.vector.scalar_tensor_tensor(
                    out=comb[:], in0=iota_sb[:], scalar=sub[:], in1=logits[:],
                    op0=ALU.add, op1=ALU.add)
                return logits, comb

            def emit_g_prefetch(t):
                tiles = []
                for b in range(NBLK):
                    g_sb = gp.tile([TILE_P, SBLK, SEG], F32, tag="g")
                    gq = SBLK // 2
                    for qq in range(2):
                        nc.sync.dma_start(
                            out=g_sb[:, qq * gq:(qq + 1) * gq, :],
                            in_=g[t, :, b * SBLK + qq * gq:b * SBLK + (qq + 1) * gq, :])
                    tiles.append(g_sb)
                return tiles

            def emit_sampling(t, logits, comb, g_tiles):
                v_sb = vall.tile([TILE_P, SAMPLE], F32, tag=f"v{t}")
                for b in range(NBLK):
                    g_sb = g_tiles[b]
                    tmp = sampw.tile([TILE_P, SBLK, SEG], F32, tag="tmp")
                    l_b = _bcast(logits[:], 1, SBLK)
                    nc.gpsimd.tensor_tensor(out=tmp[:], in0=g_sb[:], in1=l_b,
                                            op=ALU.add)
                    negmx = mxp.tile([TILE_P, SBLK], F32, tag="negmx")
                    nc.vector.tensor_reduce(out=negmx[:], in_=tmp[:], axis=AX.X,
                                            op=ALU.max, negate=True)
                    for sl in range(SBLK):
                        nc.scalar.activation(tmp[:, sl, :], tmp[:, sl, :],
                                             ACTF.Sign,
                                             bias=negmx[:, sl:sl + 1])
                    c_b = _bcast(comb[:], 1, SBLK)
                    nc.vector.scalar_tensor_tensor(
                        out=tmp[:], in0=tmp[:], scalar=0.0, in1=c_b,
                        op0=ALU.is_ge, op1=ALU.mult)
                    nc.vector.reduce_sum(out=v_sb[:, b * SBLK:(b + 1) * SBLK],
                                         in_=tmp[:], axis=AX.X)
                nc.scalar.dma_start(out=v_out[t], in_=v_sb[:])

            raw0 = emit_mm2(0)
            logits0, comb0 = emit_pipe(0, raw0)
            g0 = emit_g_prefetch(0)
            raw1 = emit_mm2(1)
            emit_sampling(0, logits0, comb0, g0)
            logits1, comb1 = emit_pipe(1, raw1)
            g1 = emit_g_prefetch(1)
            emit_sampling(1, logits1, comb1, g1)

    nc.compile()
    return nc


def _get_graph():
    if "nc" not in _cache:
        _cache["nc"] = _build_graph()
    return _cache["nc"]


def kernel(inputs, W1, b1, W2, b2, num_service):
    inputs = np.ascontiguousarray(np.asarray(inputs, dtype=np.float32))
    W1 = np.asarray(W1, dtype=np.float32)
    b1v = np.asarray(b1, dtype=np.float32).reshape(HIDDEN, 1)
    W2 = np.asarray(W2, dtype=np.float32)
    b2v = np.asarray(b2, dtype=np.float32)

    G = _gumbel_const()
    if "Gn" not in _cache:
        _cache["Gn"] = np.ascontiguousarray(np.transpose(G, (1, 0, 2)))
    Gn = _cache["Gn"]

    iota_host = np.tile((POS_SCALE * np.arange(SEG, dtype=np.float32))[None, :],
                        (TILE_P, 1))
    xw_host = np.ascontiguousarray(inputs.reshape(NKF, 128).T)   # [128, NKF]
    # W1 [16384,128] -> [128 lane, 128 chunk, 128 col], contiguous per lane
    w1t_host = np.ascontiguousarray(
        W1.reshape(NKF, 128, HIDDEN).transpose(1, 0, 2))

    in_maps = []
    for c in range(N_CORES):
        in_maps.append({
            "xw": xw_host,
            "w1": w1t_host,
            "b1": b1v,
            "w2": np.ascontiguousarray(W2[:, c * NPC:(c + 1) * NPC]),
            "b2t": np.ascontiguousarray(
                b2v[c * NPC:(c + 1) * NPC].reshape(NTILES, TILE_P, SEG)),
            "iota1000": iota_host,
            "g": np.ascontiguousarray(
                Gn[c * SEGS_PC:(c + 1) * SEGS_PC].reshape(NTILES, TILE_P, SAMPLE, SEG)),
        })

    nc = _get_graph()
    trace = bool(int(os.environ.get("KERNEL_TRACE", "0")))
    res = run_bass_kernel_spmd(nc, in_maps, core_ids=list(range(N_CORES)),
                               trace=trace)
    if trace and res.exec_time_ns is not None:
        print(f"HW exec time: {res.exec_time_ns} ns")
        _cache["exec_time_ns"] = res.exec_time_ns
        _cache["results"] = res

    scores = np.concatenate([np.asarray(r["scores_out"]).ravel()
                             for r in res.results])
    v = np.concatenate(
        [np.asarray(r["v_out"]).reshape(SEGS_PC, SAMPLE) for r in res.results],
        axis=0)                                    # [2000, 128]
    v = v.T                                        # [sample, node]
    pos = np.round(v / POS_SCALE).astype(np.int32)  # exact: |lsm| < 25 << 500
    lsm_p = v.astype(np.float64) - POS_SCALE * pos
    log_softmax = np.sum(lsm_p, axis=1).astype(np.float32)
    return pos, log_softmax, scores
